# revision 29
# baseline (speedup 1.0000x reference)
"""GCN classifier kernel for Trainium2 (Bass/Tile), 8-core SPMD.

Math: for each GCN layer, relu(nd * (A^T (ns * h)) @ W + b)
  == relu(nd_dst * sum_e p[src_e] + b) aggregated per dst, where
  p = ns * (h @ W) (ns folded into the table rows) and the one-hot
  scatter matrix S is pure 0/1 (no per-edge weight).
Layer 0 input h0 = in_deg (rank 1), so layer 1 collapses to
  h1 = relu(q1[:, None] * W0 + b0) with q1 host-precomputable from the
  graph alone.

Nodes are sharded at GRAPH boundaries (graph_ids is sorted), so every
graph's nodes live on exactly one core: the per-graph readout is
core-local and needs NO AllReduce — the host assembles the output from
the 8 per-core results. Only the two AllGathers (layer tables) remain
as collectives.

Device pipeline per core (owns <= NPC dst nodes = BLOCKS blocks of 128):
  L1: h1T = relu(W0^T q1 + b0) per block; p1 = ns * (h1 @ W1) -> slab1
  AllGather slab1 -> table1 (replicated NPAD x 128 bf16)
  L2: dma_gather msgs = table1[src] (bf16); agg[dst,f] += S_chunk^T @ msg
      (PE, bf16); h2 = relu(nd*agg + b1) -> transpose -> p2 -> slab2
  AllGather slab2 -> table2
  L3: same agg; h3 = relu(nd*agg + b2) (fp16)
      readout: rall[feat, graph] += h3^T @ Sg4 (fp16, one 512-wide psum)
  head (local): out[g, c] = invc[g] * (rall^T Wc)[g, c] + bc[c]

S chunks are one-hot(dst) matrices generated on-device by DVE
tensor_tensor(is_equal) from host-prepared per-chunk columns (bf16,
duplicated pairs so the 2x DVE mode engages). The layer bias is folded
into the aggregation as one constant matmul chunk (S_bias[0,d]=1/nd[d]).
dma_gather indices are int16, so the NPAD-row table is addressed via two
overlapping base windows (rows 0..32767 and NPAD-32768..NPAD-1).
"""

import sys

sys.path.insert(0, "/opt/trn_rl_repo")

import numpy as np

import concourse.bass as bass
import concourse.mybir as mybir
import concourse.tile as tile
from concourse import bacc, bass_utils

P = 128
N_CORES = 8
N_NODES = 50000
N_EDGES = 800000
HID = 128
N_GRAPHS = 512
N_CLASSES = 10

HALF0 = 32768       # gather window 0: rows [0, 32768)
GA = 8              # gather group size in chunks of 128 edges (1024 idxs = HW cap per dma_gather)
F32 = mybir.dt.float32
BF16 = mybir.dt.bfloat16
F16 = mybir.dt.float16
I16 = mybir.dt.int16
I32 = mybir.dt.int32
NP_BF16 = mybir.dt.np(BF16)


def _prep_graph(src, dst, graph_ids):
    """Host-side preprocessing: degrees, q1, per-core edge schedule."""
    src = np.asarray(src).astype(np.int64)
    dst = np.asarray(dst).astype(np.int64)
    graph_ids = np.asarray(graph_ids).astype(np.int64)

    in_deg = np.bincount(dst, minlength=N_NODES).astype(np.float32)
    out_deg = np.bincount(src, minlength=N_NODES).astype(np.float32)
    ns = np.maximum(out_deg, 1.0) ** -0.5
    nd = np.maximum(in_deg, 1.0) ** -0.5
    # layer-1 aggregate: q1 = nd * segsum_dst((in_deg*ns)[src])
    c0 = (in_deg * ns).astype(np.float64)
    t1 = np.bincount(dst, weights=c0[src], minlength=N_NODES)
    q1 = (nd.astype(np.float64) * t1).astype(np.float32)

    # node shard boundaries aligned to graph boundaries (graph_ids sorted)
    gcnt = np.bincount(graph_ids, minlength=N_GRAPHS)
    gcum = np.concatenate([[0], np.cumsum(gcnt)])
    starts = [0]
    gstarts = [0]
    for c in range(1, N_CORES):
        target = round(c * N_NODES / N_CORES)
        gi = int(np.argmin(np.abs(gcum - target)))
        starts.append(int(gcum[gi]))
        gstarts.append(gi)
    starts.append(N_NODES)
    gstarts.append(N_GRAPHS)
    starts = np.asarray(starts, np.int64)

    NPC = int(-(-int(np.diff(starts).max()) // P) * P)
    BLOCKS = NPC // P
    NPAD = NPC * N_CORES
    BASE1 = NPAD - HALF0
    assert NPAD <= 65536 and BASE1 >= 0

    # node id -> padded table row
    core_of = np.searchsorted(starts[1:], np.arange(N_NODES), side="right")
    row_of = core_of * NPC + (np.arange(N_NODES) - starts[core_of])
    src_row = row_of[src]

    counts = np.zeros((N_CORES, BLOCKS, 2), np.int64)
    per_core = []
    for c in range(N_CORES):
        lo, hi = starts[c], starts[c + 1]
        m = (dst >= lo) & (dst < hi)
        es, ed = src_row[m], dst[m]
        dloc = ed - lo
        blk = dloc >> 7
        # edges with src row in [BASE1, HALF0) fit either gather window;
        # assign them per block to minimize chunk padding (ceil waste)
        half = (es >= HALF0).astype(np.int64)
        over = (es >= BASE1) & (es < HALF0)
        for b in range(BLOCKS):
            mb = blk == b
            n_low = int(np.count_nonzero(mb & (es < BASE1)))
            n_over = int(np.count_nonzero(mb & over))
            cands = {0, n_over}
            k = (-n_low) % P
            while k <= n_over:
                cands.add(k)
                k += P
            n_high = int(np.count_nonzero(mb & (es >= HALF0)))
            best_x, best_cost = 0, 10**9
            for x in sorted(cands):
                cost = -(-(n_low + x) // P) + -(-(n_high + n_over - x) // P)
                if cost < best_cost:
                    best_cost, best_x = cost, x
            if best_x < n_over:
                idxs_over = np.nonzero(mb & over)[0]
                half[idxs_over[best_x:]] = 1
        order = np.lexsort((es, half, blk))
        es, dloc, blk, half = es[order], dloc[order], blk[order], half[order]
        for b in range(BLOCKS):
            mb = blk == b
            counts[c, b, 0] = np.count_nonzero(mb & (half == 0))
            counts[c, b, 1] = np.count_nonzero(mb & (half == 1))
        per_core.append((es, dloc, blk, half))

    K0 = np.maximum(1, np.ceil(counts[:, :, 0] / P).max(axis=0).astype(np.int64))
    K1 = np.maximum(1, np.ceil(counts[:, :, 1] / P).max(axis=0).astype(np.int64))
    KA = int(K0.sum())
    KB = int(K1.sum())

    core_arrays = []
    for c in range(N_CORES):
        es, dloc, blk, half = per_core[c]
        lo, hi = int(starts[c]), int(starts[c + 1])
        idxA = np.zeros(KA * P, np.int32)
        dvA = np.full(KA * P, -1.0, np.float32)
        idxB = np.zeros(KB * P, np.int32)
        dvB = np.full(KB * P, -1.0, np.float32)
        offA = 0
        offB = 0
        for b in range(BLOCKS):
            for h, (idxs, dvs, K, off) in enumerate((
                (idxA, dvA, int(K0[b]), offA),
                (idxB, dvB, int(K1[b]), offB),
            )):
                m = (blk == b) & (half == h)
                n = int(np.count_nonzero(m))
                assert n <= K * P
                sl = slice(off, off + n)
                idxs[sl] = es[m] - (0 if h == 0 else BASE1)
                dvs[sl] = (dloc[m] - b * P).astype(np.float32)
                # padding: idx=0 (gathers row 0), dv=-1 (is_equal never
                # matches any iota column) -> contributes nothing
            offA += int(K0[b]) * P
            offB += int(K1[b]) * P

        def idx_layout(v):
            # index i -> partition i%16 (replicated x8), column i//16
            r = v.astype(np.int16).reshape(-1, 16).T  # [16, L/16]
            return np.tile(r, (8, 1)).copy()  # [128, L/16]

        def col_layout(v):
            return np.ascontiguousarray(v.reshape(-1, P).T)  # [128, K]

        def dup2(a):
            # [P, K] -> [P, 2K] with each column duplicated (for DVE 2x-mode
            # views whose last AP dim must be stride-1 count-2)
            return np.ascontiguousarray(np.repeat(a, 2, axis=1))

        own = np.arange(lo, lo + NPC)
        real = own < hi
        q1row = np.zeros((1, NPC), np.float32)
        q1row[0, real] = q1[own[real]]
        gph = np.full(NPC, -1.0, np.float32)
        gph[real] = graph_ids[own[real]].astype(np.float32)
        nsv = np.ones(NPC, np.float32)
        nsv[real] = ns[own[real]]
        ndv = np.ones(NPC, np.float32)
        ndv[real] = nd[own[real]]

        # bias chunk: agg += S_bias^T @ bch with S_bias[0, d] = 1/nd[d]
        ibias = np.zeros((P, NPC), np.float32)
        ibias[0, :] = 1.0 / ndv

        core_arrays.append(dict(
            idxA=idx_layout(idxA), idxB=idx_layout(idxB),
            dvA=dup2(col_layout(dvA)).astype(NP_BF16),
            dvB=dup2(col_layout(dvB)).astype(NP_BF16),
            q1row=q1row.astype(NP_BF16),
            gphv=dup2(np.ascontiguousarray(
                gph.reshape(BLOCKS, P).T)).astype(np.float16),
            nsv=np.ascontiguousarray(nsv.reshape(BLOCKS, P).T),
            ndv=np.ascontiguousarray(ndv.reshape(BLOCKS, P).T),
            ibias=ibias.astype(NP_BF16),
        ))

    cnt = np.bincount(graph_ids, minlength=N_GRAPHS).astype(np.float32)
    invc = (1.0 / np.maximum(cnt, 1.0)).reshape(N_GRAPHS // P, P).T  # [128, 4]
    invc = np.ascontiguousarray(invc)

    sched = dict(K0=K0, K1=K1, KA=KA, KB=KB, NPC=NPC, BLOCKS=BLOCKS,
                 NPAD=NPAD, BASE1=BASE1, gstarts=gstarts)
    return sched, core_arrays, invc


def build_nc(sched, reps=1, with_coll=3, with_gather=True,
             with_sgen=True, with_compute=True, msg_bufs=8, sgen_bufs=6,
             hbuf_bufs=6):
    """Build and compile the 8-core SPMD Bass program.

    reps>1 repeats the whole pipeline inside one NEFF (for timing via
    slope); with_coll is a bitmask (1=AG1, 2=AG2); with_gather=False etc
    drop phases (timing only — results are garbage)."""
    K0, K1, KA, KB = sched["K0"], sched["K1"], sched["KA"], sched["KB"]
    NPC, BLOCKS = sched["NPC"], sched["BLOCKS"]
    NPAD, BASE1 = sched["NPAD"], sched["BASE1"]
    NGT = N_GRAPHS // P  # 4

    nc = bacc.Bacc("TRN2", target_bir_lowering=False, debug=False,
                   num_devices=N_CORES, num_swdge_queues=4)

    def inp(name, shape, dt=F32):
        return nc.dram_tensor(name, list(shape), dt, kind="ExternalInput").ap()

    d_idxA = inp("idxA", [P, KA * 8], I16)
    d_idxB = inp("idxB", [P, KB * 8], I16)
    d_dvA = inp("dvA", [P, 2 * KA], BF16)
    d_dvB = inp("dvB", [P, 2 * KB], BF16)
    d_q1 = inp("q1row", [1, NPC], BF16)
    d_gph = inp("gphv", [P, 2 * BLOCKS], F16)
    d_nsv = inp("nsv", [P, BLOCKS])
    d_ndv = inp("ndv", [P, BLOCKS])
    d_ibias = inp("ibias", [P, NPC], BF16)
    d_invc = inp("invc", [P, NGT])
    d_W0 = inp("W0", [1, HID], BF16)
    d_W1 = inp("W1", [HID, HID], BF16)
    d_W2 = inp("W2", [HID, HID], BF16)
    d_Wc = inp("Wc", [HID, N_CLASSES])
    d_b0c = inp("b0c", [P, 1])
    d_b1ch = inp("b1ch", [P, HID], BF16)
    d_b2ch = inp("b2ch", [P, HID], BF16)
    d_bcr = inp("bcr", [P, N_CLASSES])

    out = nc.dram_tensor("out", [N_GRAPHS, N_CLASSES], F32,
                         kind="ExternalOutput").ap()

    slab1 = nc.dram_tensor("slab1", [NPC, HID], BF16, kind="Internal").ap()
    slab2 = nc.dram_tensor("slab2", [NPC, HID], BF16, kind="Internal").ap()
    table1 = nc.dram_tensor("table1", [NPAD, HID], BF16, kind="Internal",
                            addr_space="Shared").ap()
    table2 = nc.dram_tensor("table2", [NPAD, HID], BF16, kind="Internal",
                            addr_space="Shared").ap()

    RG = [list(range(N_CORES))]

    # block -> chunk ranges in streams A and B
    offA = np.concatenate([[0], np.cumsum(K0)]).astype(int)
    offB = np.concatenate([[0], np.cumsum(K1)]).astype(int)

    with tile.TileContext(nc) as tc:
        with tc.tile_pool(name="const", bufs=1) as cp, \
             tc.tile_pool(name="msg", bufs=msg_bufs) as mp, \
             tc.tile_pool(name="sgen", bufs=sgen_bufs) as sp, \
             tc.tile_pool(name="hbuf", bufs=hbuf_bufs) as hp, \
             tc.tile_pool(name="agg_ps", bufs=2, space="PSUM") as agg_ps, \
             tc.tile_pool(name="p_ps", bufs=2, space="PSUM") as p_ps, \
             tc.tile_pool(name="r_ps", bufs=1, space="PSUM") as r_ps:

            def load_const(ap_in, shape, dt=F32):
                t = cp.tile(list(shape), dt, tag=ap_in.name)
                nc.sync.dma_start(t[:], ap_in[:])
                return t

            idxA = load_const(d_idxA, [P, KA * 8], I16)
            idxB = load_const(d_idxB, [P, KB * 8], I16)
            dvA = load_const(d_dvA, [P, 2 * KA], BF16)
            dvB = load_const(d_dvB, [P, 2 * KB], BF16)
            q1r = load_const(d_q1, [1, NPC], BF16)
            gph = load_const(d_gph, [P, 2 * BLOCKS], F16)
            nsv = load_const(d_nsv, [P, BLOCKS])
            ndv = load_const(d_ndv, [P, BLOCKS])
            ibias = load_const(d_ibias, [P, NPC], BF16)
            invc = load_const(d_invc, [P, NGT])
            W0 = load_const(d_W0, [1, HID], BF16)
            W1 = load_const(d_W1, [HID, HID], BF16)
            W2 = load_const(d_W2, [HID, HID], BF16)
            Wc = load_const(d_Wc, [HID, N_CLASSES])
            b0c = load_const(d_b0c, [P, 1])
            b1ch = load_const(d_b1ch, [P, HID], BF16)
            b2ch = load_const(d_b2ch, [P, HID], BF16)
            bcr = load_const(d_bcr, [P, N_CLASSES])

            iota_i = cp.tile([P, P], I32, tag="iota_i")
            nc.gpsimd.iota(iota_i[:], pattern=[[1, P]], base=0,
                           channel_multiplier=0)
            iota_f = cp.tile([P, P], BF16, tag="iota_f")
            nc.vector.tensor_copy(iota_f[:], iota_i[:])
            iotg_i = cp.tile([P, N_GRAPHS], I32, tag="iotg_i")
            nc.gpsimd.iota(iotg_i[:], pattern=[[1, N_GRAPHS]], base=0,
                           channel_multiplier=0)
            iotg_f = cp.tile([P, N_GRAPHS], F16, tag="iotg_f")
            nc.vector.tensor_copy(iotg_f[:], iotg_i[:])

            identB = cp.tile([P, P], BF16, tag="identB")
            from concourse.masks import make_identity
            make_identity(nc, identB[:])

            RELU = mybir.ActivationFunctionType.Relu
            COPY = mybir.ActivationFunctionType.Copy

            # block -> chunk list over both streams
            def block_chunks(b):
                res = []
                for ca in range(offA[b], offA[b + 1]):
                    res.append(("A", ca))
                for cb in range(offB[b], offB[b + 1]):
                    res.append(("B", cb))
                return res

            # global gather counter: buf index (mod msg_bufs) and queue
            # (mod 4) stay consistent across layers/reps since 4 | msg_bufs
            qctr = [0]

            for rep in range(reps):
                # ---------------- layer 1 ----------------
                for k in range(BLOCKS):
                    h1T_psum = agg_ps.tile([P, P], F32, tag="aggps")
                    nc.tensor.matmul(
                        out=h1T_psum[:], lhsT=W0[:],
                        rhs=q1r[:][:, k * P:(k + 1) * P],
                        start=True, stop=True)
                    h1T = hp.tile([P, P], BF16, tag="hT")
                    nc.scalar.activation(out=h1T[:], in_=h1T_psum[:],
                                         func=RELU, bias=b0c[:], scale=1.0)
                    p_psum = p_ps.tile([P, P], F32, tag="pps")
                    nc.tensor.matmul(out=p_psum[:], lhsT=h1T[:], rhs=W1[:],
                                     start=True, stop=True)
                    p_sb = hp.tile([P, P], BF16, tag="pout")
                    nc.scalar.activation(
                        out=p_sb[:], in_=p_psum[:],
                        func=COPY, bias=0.0, scale=nsv[:][:, k:k + 1])
                    nc.sync.dma_start(slab1[k * P:(k + 1) * P, :], p_sb[:])

                if with_coll & 1:
                    nc.gpsimd.collective_compute(
                        "AllGather", mybir.AluOpType.bypass,
                        replica_groups=RG,
                        ins=[slab1[:]], outs=[table1[:]])

                def emit_gathers(table_ap):
                    """Gather + S-gen per group, block-sorted across streams.

                    Returns chunk -> (msg tile, S8 tile, col). Pool executes
                    gathers in order, so groups are emitted in first-use
                    (block) order to avoid msg-slot deadlock.
                    """
                    chunk_src = {}
                    groups = []
                    blockA = np.searchsorted(offA[1:], np.arange(KA),
                                             side="right")
                    blockB = np.searchsorted(offB[1:], np.arange(KB),
                                             side="right")
                    for stream, K, idx_t, blk_of in (
                            ("A", KA, idxA, blockA), ("B", KB, idxB, blockB)):
                        base_ap = (table_ap[0:HALF0, :] if stream == "A"
                                   else table_ap[BASE1:NPAD, :])
                        g0 = 0
                        while g0 < K:
                            ln = min(GA, K - g0)
                            groups.append(
                                (int(blk_of[g0]), stream, g0, ln, base_ap,
                                 idx_t))
                            g0 += ln
                    groups.sort(key=lambda g: (g[0], g[1]))
                    for _fb, stream, g0, ln, base_ap, idx_t in groups:
                        gi = qctr[0]
                        qctr[0] += 1
                        mt = mp.tile([P, GA * P], BF16, tag="msg")
                        out_ap = mt[:][:, :ln * P].rearrange(
                            "p (a b) -> p a b", b=P)
                        if with_gather:
                            nc.gpsimd.dma_gather(
                                out_ap=out_ap, in_ap=base_ap,
                                idxs_ap=idx_t[:][:, g0 * 8:(g0 + ln) * 8],
                                num_idxs=ln * P, num_idxs_reg=ln * P,
                                elem_size=HID, queue_num=gi % 4)
                        # S for the whole group: one wide DVE is_equal.
                        # All views keep a stride-1 count-2 last dim (dv is
                        # host-duplicated) so DVE 2x-1p mode engages.
                        dv = dvA if stream == "A" else dvB
                        S8 = sp.tile([P, GA * P], BF16, tag="S8")
                        s_ap = S8[:][:, :ln * P].rearrange(
                            "p (a b two) -> p a b two", b=P // 2, two=2)
                        if with_sgen:
                            io8 = iota_f[:].rearrange(
                                "p (b two) -> p b two", two=2).unsqueeze(1). \
                                broadcast_to([P, ln, P // 2, 2])
                            dv8 = dv[:][:, 2 * g0:2 * (g0 + ln)].rearrange(
                                "p (a two) -> p a two", two=2).unsqueeze(2). \
                                broadcast_to([P, ln, P // 2, 2])
                            nc.vector.tensor_tensor(
                                out=s_ap, in0=io8, in1=dv8,
                                op=mybir.AluOpType.is_equal)
                        for j in range(ln):
                            chunk_src[(stream, g0 + j)] = (mt, S8, j)
                    return chunk_src

                def agg_layer(table_ap, last):
                    """Aggregate via one-hot matmuls; relu(nd*agg + b)."""
                    chunk_src = emit_gathers(table_ap)
                    if last:
                        # rall[feat, graph] accumulates h3^T @ Sg4 per block
                        rall = r_ps.tile([P, N_GRAPHS], F32, tag="rall",
                                         name=f"rall_{rep}")
                    for b in range(BLOCKS if with_compute else 0):
                        chunks = block_chunks(b)
                        agg = agg_ps.tile([P, P], F32, tag="aggps")
                        for j, (stream, ci) in enumerate(chunks):
                            mt, S8, col = chunk_src[(stream, ci)]
                            nc.tensor.matmul(
                                out=agg[:],
                                lhsT=S8[:][:, col * P:(col + 1) * P],
                                rhs=mt[:][:, col * P:(col + 1) * P],
                                start=(j == 0), stop=False)
                        # bias chunk: agg += S_bias^T @ b_ch where
                        # S_bias[0, d] = 1/nd[d]; the activation below then
                        # computes relu(nd*agg + b) in one pass.
                        nc.tensor.matmul(
                            out=agg[:],
                            lhsT=ibias[:][:, b * P:(b + 1) * P],
                            rhs=(b2ch if last else b1ch)[:],
                            start=False, stop=True)
                        nd_col = ndv[:][:, b:b + 1]
                        if not last:
                            h2 = hp.tile([P, P], BF16, tag="h2")
                            nc.scalar.activation(out=h2[:], in_=agg[:],
                                                 func=RELU, bias=0.0,
                                                 scale=nd_col)
                            # transpose h2 -> h2T for the p-matmul
                            h2T_ps = p_ps.tile([P, P], BF16, tag="tps")
                            nc.tensor.transpose(out=h2T_ps[:], in_=h2[:],
                                                identity=identB[:])
                            h2T = hp.tile([P, P], BF16, tag="hT")
                            nc.vector.tensor_copy(h2T[:], h2T_ps[:])
                            p_psum = p_ps.tile([P, P], F32, tag="pps")
                            nc.tensor.matmul(out=p_psum[:], lhsT=h2T[:],
                                             rhs=W2[:], start=True, stop=True)
                            p_sb = hp.tile([P, P], BF16, tag="pout")
                            nc.scalar.activation(
                                out=p_sb[:], in_=p_psum[:], func=COPY,
                                bias=0.0, scale=nsv[:][:, b:b + 1])
                            nc.sync.dma_start(slab2[b * P:(b + 1) * P, :],
                                              p_sb[:])
                        else:
                            h3 = hp.tile([P, P], F16, tag="h3")
                            nc.scalar.activation(out=h3[:], in_=agg[:],
                                                 func=RELU, bias=0.0,
                                                 scale=nd_col)
                            # readout: one wide one-hot over 4 graph tiles
                            # (gph host-duplicated for the 2x-mode view)
                            Sg4 = sp.tile([P, N_GRAPHS], F16, tag="Sg4")
                            sg_ap = Sg4[:].rearrange(
                                "p (b two) -> p b two", two=2)
                            nc.vector.tensor_tensor(
                                out=sg_ap,
                                in0=iotg_f[:].rearrange(
                                    "p (b two) -> p b two", two=2),
                                in1=gph[:][:, 2 * b:2 * b + 2].unsqueeze(1).
                                broadcast_to([P, N_GRAPHS // 2, 2]),
                                op=mybir.AluOpType.is_equal)
                            nc.tensor.matmul(
                                out=rall[:], lhsT=h3[:], rhs=Sg4[:],
                                start=(b == 0), stop=(b == BLOCKS - 1))
                    if last and with_compute:
                        r_sb = hp.tile([P, N_GRAPHS], F32, tag="rsb")
                        nc.vector.tensor_copy(r_sb[:], rall[:])
                        # ------------ head (local; no AllReduce) ------------
                        # o[g, c] = invc[g] * sum_f r_sb[f, g] Wc[f, c] + bc
                        for t in range(NGT):
                            o_psum = p_ps.tile([P, P], F32, tag="pps")
                            nc.tensor.matmul(
                                out=o_psum[:][:, :N_CLASSES],
                                lhsT=r_sb[:][:, t * P:(t + 1) * P],
                                rhs=Wc[:], start=True, stop=True)
                            o_sc = hp.tile([P, N_CLASSES], F32, tag="osc")
                            nc.vector.tensor_scalar(
                                out=o_sc[:], in0=o_psum[:][:, :N_CLASSES],
                                scalar1=invc[:][:, t:t + 1], scalar2=None,
                                op0=mybir.AluOpType.mult)
                            o_sb = hp.tile([P, N_CLASSES], F32, tag="osb")
                            nc.vector.tensor_tensor(
                                out=o_sb[:], in0=o_sc[:], in1=bcr[:],
                                op=mybir.AluOpType.add)
                            nc.sync.dma_start(out[t * P:(t + 1) * P, :],
                                              o_sb[:])

                # ---------------- layer 2 ----------------
                agg_layer(table1, last=False)

                if with_coll & 2:
                    nc.gpsimd.collective_compute(
                        "AllGather", mybir.AluOpType.bypass,
                        replica_groups=RG,
                        ins=[slab2[:]], outs=[table2[:]])

                # ---------------- layer 3 + readout + head ----------------
                agg_layer(table2, last=True)

    nc.compile()
    return nc


def make_in_maps(core_arrays, invc, W0, b0, W1, b1, W2, b2, Wc, bc):
    common = dict(
        invc=np.ascontiguousarray(invc, np.float32),
        W0=np.ascontiguousarray(W0, np.float32).reshape(1, HID).astype(NP_BF16),
        W1=np.ascontiguousarray(W1, np.float32).astype(NP_BF16),
        W2=np.ascontiguousarray(W2, np.float32).astype(NP_BF16),
        Wc=np.ascontiguousarray(Wc, np.float32),
        b0c=np.ascontiguousarray(b0, np.float32).reshape(P, 1),
        b1ch=_bias_chunk(b1),
        b2ch=_bias_chunk(b2),
        bcr=np.ascontiguousarray(np.tile(
            np.asarray(bc, np.float32).reshape(1, N_CLASSES), (P, 1))),
    )
    in_maps = []
    for c in range(N_CORES):
        m = dict(common)
        ca = core_arrays[c]
        for k in ("idxA", "idxB", "dvA", "dvB", "q1row", "gphv", "nsv",
                  "ndv", "ibias"):
            m[k] = ca[k]
        in_maps.append(m)
    return in_maps


def _bias_chunk(b):
    ch = np.zeros((P, HID), np.float32)
    ch[0, :] = np.asarray(b, np.float32)
    return ch.astype(NP_BF16)


_CACHE = {}


def _get_compiled(src, dst, graph_ids):
    import hashlib
    h = hashlib.md5()
    h.update(np.asarray(src).tobytes())
    h.update(np.asarray(dst).tobytes())
    h.update(np.asarray(graph_ids).tobytes())
    key = h.hexdigest()
    if key not in _CACHE:
        sched, core_arrays, invc = _prep_graph(src, dst, graph_ids)
        nc = build_nc(sched)
        _CACHE[key] = (sched, nc, core_arrays, invc)
    return _CACHE[key]


def kernel(W0, b0, W1, b1, W2, b2, Wc, bc, src, dst, graph_ids,
           num_graphs=None, **_ignored):
    sched, nc, core_arrays, invc = _get_compiled(src, dst, graph_ids)
    in_maps = make_in_maps(core_arrays, invc, W0, b0, W1, b1, W2, b2, Wc, bc)
    res = bass_utils.run_bass_kernel_spmd(
        nc, in_maps, core_ids=list(range(N_CORES)))
    # each core's output rows are valid only for its own graphs — assemble
    gstarts = sched["gstarts"]
    full = np.zeros((N_GRAPHS, N_CLASSES), np.float32)
    for c in range(N_CORES):
        g0, g1 = gstarts[c], gstarts[c + 1]
        full[g0:g1] = res.results[c]["out"][g0:g1]
    return full
